# revision 52
# baseline (speedup 1.0000x reference)
"""GQA attention forward (dense_transformer) on 8 TRN2 NeuronCores.

Problem: x[2,2048,2048] -> RoPE'd GQA attention (16 q-heads, 4 kv-heads,
head_dim 128, causal) -> out @ Wo, f32.

Sharding: core = (batch b, kv-head g). Each core handles one batch and one
kv-group (4 q-heads + its kv head): computes q/k/v projections for its
columns of Wq/Wk/Wv, attention for its 4 heads, and a partial output
through its 512 rows of Wo. Host sums the 4 partials per batch (bf16
partials, f32 accumulate).

On-device layout (host-side transforms):
 - x is transposed on host (xT [D, S]) so the model dim (contraction dim of
   the QKV projections) lands on SBUF partitions.
 - Wq/Wk columns are permuted per head so RoPE pairs are de-interleaved to
   [real(64) | imag(64)]; scores are permutation-invariant since q and k are
   permuted identically. 1/sqrt(head_dim) is folded into Wq.
 - Projections produce qT/kT/vT [head_dim, S] directly (weights stationary,
   xT moving).
 - RoPE in T-layout: out = q*A + swap64(q*Bsw), where swap64 is a
   partition-half swap done with a tiny SBUF->SBUF DMA; A/Bsw are host-built
   [128, S] tables.
 - Attention is computed transposed: scoresT[k_row, q_row] = kT.T @ qT,
   exp on ScalarE (no max subtraction needed: |scores| <= ~9.3 by
   Cauchy-Schwarz on these magnitudes), bf16 probs.
 - Causality: diagonal k-tiles only need q columns >= 128*m (m = tile index
   within the group), so scores/exp/PV/rowsum all run on partial windows;
   the triangular [128,128] sub-block is masked with one resident tri mask.
 - Softmax denominator: a ones[128,128] stationary matmul accumulates
   sum_k(et) broadcast across all 128 partitions, so normalization is a
   full-width DVE reciprocal + multiply (no [1,N] ops, no bf16 rounding of
   the denominator).
 - Final: out[q_row, :] = sum_h oT_h.T @ Wo_h with q_row on partitions,
   bf16 partial written to DRAM.
"""

import os

import numpy as np
import ml_dtypes

import concourse.bass as bass
import concourse.bass_isa as bass_isa
import concourse.bacc as bacc_mod
import concourse.mybir as mybir
import concourse.tile as tile
from concourse.bass_utils import run_bass_kernel_spmd

# Model constants (hardcoded per harness contract)
DIM = 2048
N_HEADS = 16
N_KV_HEADS = 4
HEAD_DIM = 128
N_REP = 4
SEQ = 2048
BATCH = 2

P = 128
KSUB = DIM // P          # 16 contraction subtiles for projections
NQH = N_REP              # 4 q heads per core
QD = NQH * HEAD_DIM      # 512 q dims per core
NQG = 4                  # 512-row groups per batch
QG = SEQ // NQG          # 512
SEQT = SEQ // P          # 16 seq tiles of 128

F32 = mybir.dt.float32
BF16 = mybir.dt.bfloat16

LAST_RESULTS = None  # stash of BassKernelResults for test harness


def build_nc():
    nc = bacc_mod.Bacc("TRN2", target_bir_lowering=False)
    # all pre-arranged host-side into SBUF layout (partition-major) so each
    # loads with one large fully-contiguous DMA
    xT = nc.dram_tensor("xT", [P, NQG, KSUB, QG], BF16, kind="ExternalInput")
    wq = nc.dram_tensor("wq", [P, NQH, KSUB, P], BF16, kind="ExternalInput")
    wkv = nc.dram_tensor("wkv", [P, KSUB, 2 * HEAD_DIM], BF16,
                         kind="ExternalInput")
    wo = nc.dram_tensor("wo", [P, NQH, DIM], BF16, kind="ExternalInput")
    ropeA = nc.dram_tensor("ropeA", [P, SEQ], BF16, kind="ExternalInput")
    ropeB = nc.dram_tensor("ropeB", [P, SEQ], BF16, kind="ExternalInput")
    tri = nc.dram_tensor("tri", [P, P], BF16, kind="ExternalInput")
    out = nc.dram_tensor("out", [SEQ, DIM], BF16, kind="ExternalOutput")

    with tile.TileContext(nc) as tc:
        with (
            tc.tile_pool(name="consts", bufs=1) as consts,
            tc.tile_pool(name="rope", bufs=2) as mpool,
            tc.tile_pool(name="qrope", bufs=2) as qpool,
            tc.tile_pool(name="exp", bufs=6) as epool,
            tc.tile_pool(name="norm", bufs=3) as npool,
            tc.tile_pool(name="outp", bufs=3) as opool,
            tc.tile_pool(name="ps_q", bufs=2, space="PSUM") as ps_q,
            tc.tile_pool(name="ps_sc", bufs=3, space="PSUM") as ps_sc,
            tc.tile_pool(name="ps_o", bufs=3, space="PSUM") as ps_o,
        ):
            # ---- resident tensors ----
            x_sb = consts.tile([P, NQG, KSUB, QG], BF16)  # full xT (8MB)
            wq_sb = consts.tile([P, NQH, KSUB, P], BF16)
            wkv_sb = consts.tile([P, KSUB, 2 * HEAD_DIM], BF16)
            wo_sb = consts.tile([P, NQH, DIM], BF16)
            A_sb = consts.tile([P, SEQ], BF16)
            B_sb = consts.tile([P, SEQ], BF16)
            tri_sb = consts.tile([P, P], BF16)
            ones_sq = consts.tile([P, P], BF16)
            nc.vector.memset(ones_sq, 1.0)

            kT_sb = consts.tile([P, SEQ], BF16)       # roped kT
            v_sb = consts.tile([P, SEQT, HEAD_DIM], BF16)  # v natural [krow,d]
            oT_sb = consts.tile([P, NQH, SEQ], BF16)  # normalized attn outT

            # rope tables + mask early on the (otherwise idle) gpsimd queue
            nc.gpsimd.dma_start(A_sb, ropeA[:, :])
            nc.gpsimd.dma_start(B_sb, ropeB[:, :])
            nc.gpsimd.dma_start(tri_sb, tri[:, :])
            # preload the exp table set during the first projections
            warm = npool.tile([P, 1], F32, tag="warm")
            nc.scalar.activation(warm, ones_sq[:, 0:1],
                                 mybir.ActivationFunctionType.Exp)

            def rope_muls(src_ps, rows):
                m1 = mpool.tile([P, QG], F32, tag="m1", name="m1")
                m2 = mpool.tile([P, QG], F32, tag="m2", name="m2")
                m2s = mpool.tile([P, QG], F32, tag="m2s", name="m2s")
                nc.vector.tensor_mul(m1, src_ps, A_sb[:, rows])
                nc.vector.tensor_mul(m2, src_ps, B_sb[:, rows])
                nc.sync.dma_start(m2s[0:64, :], m2[64:128, :])
                nc.sync.dma_start(m2s[64:128, :], m2[0:64, :])
                return m1, m2s

            def rope(src_ps, dst, rows):
                m1, m2s = rope_muls(src_ps, rows)
                nc.vector.tensor_add(dst, m1, m2s)

            def proj(w_slice, qg, q_out):
                for k in range(KSUB):
                    nc.tensor.matmul(
                        q_out, w_slice(k), x_sb[:, qg, k, :],
                        start=(k == 0), stop=(k == KSUB - 1))

            def kv_proj(qg):
                # K in T-layout (w stationary); V directly in natural
                # [seq, d] layout (x-tile stationary) — no transposes needed
                k_ps = ps_q.tile([P, QG], F32, tag="q", name="k_ps")
                for k in range(KSUB):
                    nc.tensor.matmul(k_ps, wkv_sb[:, k, 0:P],
                                     x_sb[:, qg, k, :],
                                     start=(k == 0), stop=(k == KSUB - 1))
                v_ps = ps_q.tile([P, QG // P, HEAD_DIM], F32, tag="q",
                                 name="v_ps")
                # start=True clears has_written for the WHOLE bank, so only
                # the very first matmul may set it; later j's first write
                # lands via overwrite-where-bit-clear.
                for k in range(KSUB):
                    for j in range(QG // P):
                        nc.tensor.matmul(
                            v_ps[:, j, :],
                            x_sb[:, qg, k, j * P:(j + 1) * P],
                            wkv_sb[:, k, P:2 * P],
                            start=(k == 0 and j == 0),
                            stop=(k == KSUB - 1 and j == QG // P - 1),
                            skip_group_check=True)
                return k_ps, v_ps

            def kv_finish(qg, k_ps, v_ps):
                rows = slice(qg * QG, (qg + 1) * QG)
                rope(k_ps, kT_sb[:, rows], rows)
                nc.vector.tensor_copy(
                    v_sb[:, qg * (QG // P):(qg + 1) * (QG // P), :], v_ps)

            def load_x(qg):
                nc.sync.dma_start(x_sb[:, qg], xT[:, qg])

            def qproj_rope(qg, h):
                rows = slice(qg * QG, (qg + 1) * QG)
                q_ps = ps_q.tile([P, QG], F32, tag="q", name="q_ps")
                proj(lambda k, h=h: wq_sb[:, h, k, :], qg, q_ps)
                qro = qpool.tile([P, QG], BF16, tag="qro", name="qro")
                rope(q_ps, qro, rows)
                return qro

            # ---- prologue ----
            # Ordered by first use; SDMA round-robins across rings so issue
            # order ~= bandwidth priority. x1-3 and wo are deferred to
            # mid-group to keep startup bandwidth for the critical loads.
            for c in range(4):  # wkv + x group 0 chunked so kv_proj streams
                ks = slice(4 * c, 4 * (c + 1))
                nc.scalar.dma_start(wkv_sb[:, ks], wkv[:, ks])
                nc.sync.dma_start(x_sb[:, 0, ks], xT[:, 0, ks])
            nc.sync.dma_start(wq_sb[:, 0], wq[:, 0])
            kv_pend = kv_proj(0)
            # gate wq1-3 issue behind the K projection so the startup DMA
            # bandwidth goes to wkv/x0/wq0 (the ACT ring is FIFO)
            gate = npool.tile([P, 1], F32, tag="warm")
            nc.scalar.copy(gate, kv_pend[0][:, 0:1])
            nc.scalar.dma_start(wq_sb[:, 1], wq[:, 1])
            nc.scalar.dma_start(wq_sb[:, 2], wq[:, 2])
            nc.scalar.dma_start(wq_sb[:, 3], wq[:, 3])
            kv_finish(0, *kv_pend)
            qro_pend = qproj_rope(0, 0)

            def boundary(qg):
                # next group's K/V proj + ropes; issued before the last
                # head's attention so the DVE/DMA chains hide under PE work.
                # rope mul/add phases are split so both partition-swap DMAs
                # are in flight together.
                rows2 = slice((qg + 1) * QG, (qg + 2) * QG)
                k_ps, v_ps = kv_proj(qg + 1)
                km = rope_muls(k_ps, rows2)
                q_ps = ps_q.tile([P, QG], F32, tag="q", name="q_ps")
                proj(lambda k: wq_sb[:, 0, k, :], qg + 1, q_ps)
                nc.vector.tensor_copy(
                    v_sb[:, (qg + 1) * (QG // P):(qg + 2) * (QG // P), :],
                    v_ps)
                qm = rope_muls(q_ps, rows2)
                nc.vector.tensor_add(kT_sb[:, rows2], *km)
                qro_n = qpool.tile([P, QG], BF16, tag="qro", name="qro")
                nc.vector.tensor_add(qro_n, *qm)
                return qro_n

            for qg in range(NQG):
                rows = slice(qg * QG, (qg + 1) * QG)
                nkt = (qg + 1) * (QG // P)
                qro = qro_pend
                norm_pend = None
                for h in range(NQH):
                    if h == 0 and qg < NQG - 1:  # prefetch next group's x
                        load_x(qg + 1)
                    if h == 1 and qg == 0:
                        nc.gpsimd.dma_start(wo_sb[:, :, :], wo[:, :, :])
                    if h == NQH - 1 and qg < NQG - 1:
                        qro_pend = boundary(qg)
                    if h < NQH - 1:  # next head's proj + rope ahead of attn
                        qro_next = qproj_rope(qg, h + 1)
                    o_ps = ps_o.tile([P, QG], F32, tag="o", name="o_ps")
                    esum = epool.tile([P, QG], BF16, tag="esum", name="esum")
                    ndiag = 4 * qg  # tiles before the diagonal band
                    LA = 3  # score/exp lookahead depth ahead of PV
                    ets = []
                    for g in range(nkt + LA):
                        if g < nkt:
                            # diagonal tiles: only q columns >= 128*m matter
                            m = g - ndiag
                            w0 = m * P if m >= 0 else 0
                            win = slice(w0, QG)
                            sc_ps = ps_sc.tile([P, QG], F32, tag="sc",
                                               name="sc_ps")
                            nc.tensor.matmul(
                                sc_ps[:, win], kT_sb[:, g * P:(g + 1) * P],
                                qro[:, win], start=True, stop=True)
                            et = epool.tile([P, QG], BF16, tag="et", name="et")
                            nc.scalar.activation(
                                et[:, win], sc_ps[:, win],
                                mybir.ActivationFunctionType.Exp)
                            if m >= 0:  # mask the triangular 128-block
                                nc.vector.tensor_mul(
                                    et[:, w0:w0 + P], et[:, w0:w0 + P],
                                    tri_sb)
                            # rowsum accumulation on DVE (partition-dim
                            # all-reduce happens once per head on GpSimd)
                            if g == 0:
                                nc.vector.tensor_copy(esum, et)
                            else:
                                nc.vector.tensor_add(
                                    esum[:, win], esum[:, win], et[:, win])
                            ets.append((et, w0))
                        if g >= LA:
                            et, w0 = ets[g - LA]
                            win = slice(w0, QG)
                            kt = g - LA
                            first, last = kt == 0, kt == nkt - 1
                            nc.tensor.matmul(o_ps[:, win], v_sb[:, kt, :],
                                             et[:, win],
                                             start=first, stop=last)
                    rs_bc = npool.tile([P, QG], F32, tag="rsbc", name="rsbc")
                    nc.gpsimd.partition_all_reduce(
                        rs_bc, esum, channels=P,
                        reduce_op=bass_isa.ReduceOp.add)
                    if norm_pend is not None:
                        norm_pend()  # previous head: deps long since ready
                    def norm_fn(h=h, o_ps=o_ps, rs_bc=rs_bc):
                        rcp = npool.tile([P, QG], F32, tag="rcp", name="rcp")
                        nc.vector.reciprocal_approx_fast(rcp, rs_bc)
                        nc.vector.tensor_mul(oT_sb[:, h, rows], o_ps, rcp)
                    norm_pend = norm_fn
                    qro = qro_next
                norm_pend()
                # output projection for this row group
                for qt in range(4 * qg, 4 * (qg + 1)):
                    qsl = slice(qt * P, (qt + 1) * P)
                    oc = opool.tile([P, DIM], BF16, tag="oc", name="oc")
                    for n in range(4):
                        wo_ch = ps_o.tile([P, QG], F32, tag="o",
                                          name="wo_ch")
                        for h in range(NQH):
                            nc.tensor.matmul(
                                wo_ch, oT_sb[:, h, qsl],
                                wo_sb[:, h, n * QG:(n + 1) * QG],
                                start=(h == 0), stop=(h == NQH - 1))
                        nc.vector.tensor_copy(oc[:, n * QG:(n + 1) * QG],
                                              wo_ch)
                    nc.gpsimd.dma_start(out[qsl, :], oc)
    nc.compile()
    return nc


_nc_cache = None


def _get_nc():
    global _nc_cache
    if _nc_cache is None:
        _nc_cache = build_nc()
    return _nc_cache


def _host_prep(x, freqs_cos, freqs_sin, Wq, Wk, Wv, Wo):
    x = np.asarray(x, dtype=np.float32)
    cos = np.asarray(freqs_cos, dtype=np.float32)
    sin = np.asarray(freqs_sin, dtype=np.float32)
    Wq = np.asarray(Wq, dtype=np.float32)
    Wk = np.asarray(Wk, dtype=np.float32)
    Wv = np.asarray(Wv, dtype=np.float32)
    Wo = np.asarray(Wo, dtype=np.float32)

    perm = np.concatenate([np.arange(0, HEAD_DIM, 2), np.arange(1, HEAD_DIM, 2)])
    scale = 1.0 / np.sqrt(np.float32(HEAD_DIM))
    Wq_p = (Wq.reshape(DIM, N_HEADS, HEAD_DIM)[:, :, perm] * scale).astype(np.float32)
    Wk_p = Wk.reshape(DIM, N_KV_HEADS, HEAD_DIM)[:, :, perm]

    # rope tables in T layout (partition = de-interleaved head dim)
    A = np.concatenate([cos.T, cos.T], axis=0).astype(ml_dtypes.bfloat16)
    Bsw = np.concatenate([sin.T, -sin.T], axis=0).astype(ml_dtypes.bfloat16)

    # triangular mask for the diagonal [128,128] block: keep k <= q
    pp = np.arange(P)[:, None]
    qq = np.arange(P)[None, :]
    tri = (pp <= qq).astype(ml_dtypes.bfloat16)

    # xT pre-arranged to SBUF layout [P, NQG, KSUB, QG]:
    # xT_pre[p, qg, k, s] = x[b, qg*QG + s, k*P + p]
    xT = []
    for b in range(BATCH):
        t = x[b].reshape(NQG, QG, KSUB, P)          # [qg, s, k, p]
        t = np.ascontiguousarray(t.transpose(3, 0, 2, 1))  # [p, qg, k, s]
        xT.append(t.astype(ml_dtypes.bfloat16))

    in_maps = []
    for core in range(8):
        b, g = divmod(core, N_KV_HEADS)
        # wq pre-arranged head-major to [P, NQH, KSUB, P]:
        # wq_pre[p, h, k, c] = Wq_p[k*P + p, head 4g + h, c]
        wq_s = Wq_p[:, N_REP * g:N_REP * (g + 1), :]     # [DIM, 4, 128]
        wq_s = wq_s.reshape(KSUB, P, NQH, P)             # [k, p, h, c]
        wq_shard = np.ascontiguousarray(
            wq_s.transpose(1, 2, 0, 3)).astype(ml_dtypes.bfloat16)
        # wkv pre-arranged to [P, KSUB, 256]
        wkv_s = np.concatenate(
            [Wk_p[:, g, :], Wv[:, g * HEAD_DIM:(g + 1) * HEAD_DIM]], axis=1)
        wkv_s = wkv_s.reshape(KSUB, P, 2 * HEAD_DIM)
        wkv_shard = np.ascontiguousarray(
            wkv_s.transpose(1, 0, 2)).astype(ml_dtypes.bfloat16)
        # wo pre-arranged to [P, NQH, DIM]
        wo_s = Wo[QD * g:QD * (g + 1), :].reshape(NQH, P, DIM)
        wo_shard = np.ascontiguousarray(
            wo_s.transpose(1, 0, 2)).astype(ml_dtypes.bfloat16)
        in_maps.append({
            "xT": xT[b],
            "wq": wq_shard,
            "wkv": wkv_shard,
            "wo": wo_shard,
            "ropeA": A,
            "ropeB": Bsw,
            "tri": tri,
        })
    return in_maps


def kernel(x, freqs_cos, freqs_sin, Wq, Wk, Wv, Wo):
    global LAST_RESULTS
    in_maps = _host_prep(x, freqs_cos, freqs_sin, Wq, Wk, Wv, Wo)
    nc = _get_nc()
    trace = bool(os.environ.get("KERNEL_TRACE"))
    res = run_bass_kernel_spmd(nc, in_maps, core_ids=list(range(8)), trace=trace)
    LAST_RESULTS = res
    outs = [m["out"].astype(np.float32) for m in res.results]
    out = np.stack(
        [sum(outs[b * N_KV_HEADS:(b + 1) * N_KV_HEADS]) for b in range(BATCH)],
        axis=0)
    return out.astype(np.float32)
